# revision 24
# baseline (speedup 1.0000x reference)
"""Trainium2 Bass kernel for nn_BasicTransformerBlock_35304631173827.

Sharding: 8 cores = 4 samples x 2 sequence halves. Each core computes its
1024-token half of one sample fully locally (self-attention K/V recomputed
over the full 2048-token sample -> zero collectives). bf16 matmuls with
fp32 PSUM accumulation; LayerNorm stats, softmax and residuals in fp32.

v2 restructure vs baseline:
- h1T kept SBUF-resident; dense projections use paired 512-col PSUM chains.
- Attention: every head's AV matmul is M=65 with a ones-column in V so the
  softmax denominator falls out of the same accumulation chain (no separate
  M=1 denominator matmuls); reciprocal_approx_fast for 1/den; odd heads'
  outputs shifted to partitions 64:128 via a small SBUF->SBUF DMA.
- qc-outer / head-inner loop with o-proj + LN interleaved to keep the PE
  array busy (p-state ramp) while the scalar engine chews softmax exps.
- FF: PSUM-accumulated FF2 (full K=4096 contraction in one chain), fused
  (a+b1)*gelu(gate+b1') via scalar_tensor_tensor, output biases folded into
  the matmul chains as K=1 ones-row accumulation steps.
"""

import numpy as np
import ml_dtypes

BF16 = ml_dtypes.bfloat16

B, N, D = 4, 2048, 1024
J, CD = 256, 768
H, DH = 16, 64
INNER = 1024
FF = 4096
P = 128
KT = D // P            # 8
CKT = CD // P          # 6
TT_FULL = N // P       # 16
N_OWN = N // 2
TT_OWN = N_OWN // P    # 8
EPS = 1e-5
SC = DH ** -0.5

_CACHE = {}


def _build_program():
    import concourse.tile as tile
    from concourse import mybir, bacc
    from concourse.masks import make_identity
    from contextlib import ExitStack

    f32 = mybir.dt.float32
    bf16 = mybir.dt.bfloat16
    AF = mybir.ActivationFunctionType
    ALU = mybir.AluOpType

    nc = bacc.Bacc(None, target_bir_lowering=False)

    xf_d = nc.dram_tensor("xf", [TT_OWN, P, D], f32, kind="ExternalInput")
    xbf_d = nc.dram_tensor("xbf", [TT_FULL, P, D], bf16, kind="ExternalInput")
    tT_d = nc.dram_tensor("tT", [P, KT], bf16, kind="ExternalInput")
    nw_d = nc.dram_tensor("nw", [P, KT, 6 * D], bf16, kind="ExternalInput")
    nbc_d = nc.dram_tensor("nbc", [P, 48], f32, kind="ExternalInput")
    wq1_d = nc.dram_tensor("wq1", [P, KT, INNER], bf16, kind="ExternalInput")
    wk1_d = nc.dram_tensor("wk1", [P, KT, INNER], bf16, kind="ExternalInput")
    wv1_d = nc.dram_tensor("wv1", [P, KT, INNER], bf16, kind="ExternalInput")
    wo1_d = nc.dram_tensor("wo1", [P, KT, D], bf16, kind="ExternalInput")
    wq2_d = nc.dram_tensor("wq2", [P, KT, INNER], bf16, kind="ExternalInput")
    wk2_d = nc.dram_tensor("wk2", [P, CKT, INNER], bf16, kind="ExternalInput")
    wv2_d = nc.dram_tensor("wv2", [P, CKT, INNER], bf16, kind="ExternalInput")
    wo2_d = nc.dram_tensor("wo2", [P, KT, D], bf16, kind="ExternalInput")
    ctxT_d = nc.dram_tensor("ctxT", [P, CKT, J], bf16, kind="ExternalInput")
    brow_d = nc.dram_tensor("brow", [1, 3 * D], bf16, kind="ExternalInput")
    fb1_d = nc.dram_tensor("fb1c", [P, 64], f32, kind="ExternalInput")
    wf1_d = nc.dram_tensor("wf1", [P, KT, 2 * FF], bf16, kind="ExternalInput")
    wf2_d = nc.dram_tensor("wf2", [P, FF // P, D], bf16, kind="ExternalInput")
    y_d = nc.dram_tensor("y", [TT_OWN, P, D], f32, kind="ExternalOutput")

    # DRAM scratch
    kT_dram = nc.dram_tensor("scr_kT", [KT, P, N], bf16, kind="Internal")
    x1_dram = nc.dram_tensor("scr_x1", [TT_OWN, P, D], f32, kind="Internal")
    x2_dram = nc.dram_tensor("scr_x2", [TT_OWN, P, D], f32, kind="Internal")

    with tile.TileContext(nc) as tc, ExitStack() as es:
        konst = es.enter_context(tc.tile_pool(name="konst", bufs=1))
        xpool = es.enter_context(tc.tile_pool(name="xpool", bufs=2))
        stats = es.enter_context(tc.tile_pool(name="stats", bufs=3))
        small = es.enter_context(tc.tile_pool(name="small", bufs=2))
        wsm = es.enter_context(tc.tile_pool(name="wsm", bufs=3))
        wbig = es.enter_context(tc.tile_pool(name="wbig", bufs=2))
        stg = es.enter_context(tc.tile_pool(name="stg", bufs=2))
        hTp = es.enter_context(tc.tile_pool(name="hTp", bufs=1))
        ps_big = es.enter_context(tc.tile_pool(name="ps_big", bufs=2, space="PSUM"))
        ps_av = es.enter_context(tc.tile_pool(name="ps_av", bufs=2, space="PSUM"))
        ps_fill = es.enter_context(tc.tile_pool(name="ps_fill", bufs=2, space="PSUM"))

        # ---------------- constants ----------------
        ident = konst.tile([P, P], bf16)
        make_identity(nc, ident)
        ones = konst.tile([1, P], bf16)
        nc.vector.memset(ones[:], 1.0)
        eps_t = konst.tile([P, 1], f32)
        nc.vector.memset(eps_t[:], EPS)
        tT_sb = konst.tile([P, KT], bf16)
        nc.sync.dma_start(tT_sb[:], tT_d[:])
        nbc_sb = konst.tile([P, 48], f32)
        nc.sync.dma_start(nbc_sb[:], nbc_d[:])
        fb1_sb = konst.tile([P, 64], f32)
        nc.sync.dma_start(fb1_sb[:], fb1_d[:])
        brow_sb = konst.tile([1, 3 * D], bf16)
        nc.sync.dma_start(brow_sb[:], brow_d[:])
        cols = konst.tile([P, 48], f32)

        # ---------------- Phase 0: AdaLN embeddings ----------------
        # emb^T chunks: cols[:, cc] = (t @ nW)[cc*128 : (cc+1)*128]
        # Only the n1 group is computed up front (it gates LN1); n2/n3 are
        # deferred past the K projection to keep the cold-start DMA window
        # small.
        def emb_group(g):
            for c in range(16):
                cc = g * 16 + c
                nwt = wsm.tile([P, KT, P], bf16, tag="wstream")
                nc.sync.dma_start(nwt[:], nw_d[:, :, cc * P:(cc + 1) * P])
                ps = ps_fill.tile([P, 512], f32, tag="fill")
                for kt in range(KT):
                    nc.tensor.matmul(ps[:, 0:1], nwt[:, kt, :], tT_sb[:, kt:kt + 1],
                                     start=(kt == 0), stop=(kt == KT - 1))
                nc.vector.tensor_copy(cols[:, cc:cc + 1], ps[:, 0:1])
            sl = slice(g * 16, g * 16 + 16)
            nc.vector.tensor_add(cols[:, sl], cols[:, sl], nbc_sb[:, sl])
            nc.vector.tensor_scalar_add(cols[:, g * 16:g * 16 + 8],
                                        cols[:, g * 16:g * 16 + 8], 1.0)

        emb_group(0)

        def layernorm_tile(x_tile, n3, dst_sb, off):
            """LayerNorm + AdaLN affine on (P, D) tile -> transposed chunks
            written to dst_sb[:, c, off:off+128]."""
            bst = stats.tile([P, 2, 6], f32, tag="bnst")
            for g in range(2):
                nc.vector.bn_stats(bst[:, g, :], x_tile[:, g * 512:(g + 1) * 512])
            mv = stats.tile([P, 4], f32, tag="mv")
            nc.vector.bn_aggr(mv[:, 0:2], bst[:])
            nc.scalar.activation(mv[:, 2:3], mv[:, 1:2], AF.Sqrt, bias=eps_t[:])
            nc.vector.reciprocal(mv[:, 2:3], mv[:, 2:3])
            nc.vector.tensor_tensor(mv[:, 3:4], mv[:, 0:1], mv[:, 2:3], ALU.mult)
            nc.vector.tensor_scalar_mul(mv[:, 3:4], mv[:, 3:4], -1.0)
            xn = small.tile([P, D], bf16, tag="xn")
            nc.scalar.activation(xn[:], x_tile[:], AF.Identity,
                                 bias=mv[:, 3:4], scale=mv[:, 2:3])
            for c in range(KT):
                pt = ps_fill.tile([P, P], bf16, tag="fill", name="pt")
                nc.tensor.transpose(pt[:], xn[:, c * P:(c + 1) * P], ident[:])
                nc.vector.tensor_scalar(
                    dst_sb[:, c, off:off + P], pt[:],
                    cols[:, n3 * 16 + c:n3 * 16 + c + 1],
                    cols[:, n3 * 16 + 8 + c:n3 * 16 + 8 + c + 1],
                    ALU.mult, ALU.add)

        # outer-attention scope: qT/q2T, v tiles
        with tc.tile_pool(name="qTp", bufs=1) as qTp, \
             tc.tile_pool(name="vp", bufs=1) as vp:

            ctxT_sb = vp.tile([P, CKT, J], bf16, tag="ctx")
            nc.sync.dma_start(ctxT_sb[:], ctxT_d[:])

            # ---------------- Phase 1+2: LN1 -> h1T (SBUF); QKV ----------------
            # LN1 interleaved with V-projection in 4-tile token groups so the
            # PE array has matmul work while LN chains stream through DVE.
            with tc.tile_pool(name="h1p", bufs=1) as h1p, \
                 tc.tile_pool(name="xbp", bufs=3) as xbp:
                h1T = h1p.tile([P, KT, N], bf16, tag="h1T")
                v_sb = vp.tile([P, TT_FULL, H, DH + 1], bf16, tag="v1")
                nc.vector.memset(v_sb[:], 1.0)
                wv_sb = wbig.tile([P, KT, INNER], bf16, tag="w")
                nc.sync.dma_start(wv_sb[:], wv1_d[:])
                def v_group(grp):
                    for tt in range(grp * 4, grp * 4 + 4):
                        ps = ps_big.tile([P, 1024], f32, tag="big")
                        for nc2 in range(2):
                            for kt in range(KT):
                                nc.tensor.matmul(
                                    ps[:, nc2 * 512:(nc2 + 1) * 512],
                                    h1T[:, kt, tt * P:(tt + 1) * P],
                                    wv_sb[:, kt, nc2 * 512:(nc2 + 1) * 512],
                                    start=(kt == 0), stop=(kt == KT - 1))
                        nc.vector.tensor_copy(
                            v_sb[:, tt, :, 0:DH],
                            ps[:].rearrange("p (hh r) -> p hh r", r=DH))

                for grp in range(4):
                    for tt in range(grp * 4, grp * 4 + 4):
                        xt = xbp.tile([P, D], bf16, tag="xb")
                        nc.sync.dma_start(xt[:], xbf_d[tt])
                        layernorm_tile(xt, 0, h1T, tt * P)
                    if grp > 0:
                        v_group(grp - 1)
                v_group(3)

                # K projection (full sample) -> DRAM
                w_sb = wbig.tile([P, KT, INNER], bf16, tag="w")
                nc.sync.dma_start(w_sb[:], wk1_d[:])
                for m in range(KT):
                    for half in range(2):
                        ps = ps_big.tile([P, 1024], f32, tag="big")
                        for qc in range(2):
                            for kt in range(KT):
                                nc.tensor.matmul(
                                    ps[:, qc * 512:(qc + 1) * 512],
                                    w_sb[:, kt, m * P:(m + 1) * P],
                                    h1T[:, kt, half * 1024 + qc * 512:
                                        half * 1024 + (qc + 1) * 512],
                                    start=(kt == 0), stop=(kt == KT - 1))
                        kst = xbp.tile([P, 1024], bf16, tag="kst")
                        nc.vector.tensor_copy(kst[:], ps[:])
                        nc.sync.dma_start(
                            kT_dram[m, :, half * 1024:(half + 1) * 1024], kst[:])

                # deferred AdaLN embedding groups (needed from LN2 on)
                emb_group(1)
                emb_group(2)

                # Q projection (own half, softmax scale pre-folded into wq1)
                qT = qTp.tile([P, KT, N_OWN], bf16, tag="qT")
                w_sb = wbig.tile([P, KT, INNER], bf16, tag="w")
                nc.sync.dma_start(w_sb[:], wq1_d[:])
                for m in range(KT):
                    ps = ps_big.tile([P, 1024], f32, tag="big")
                    for qc in range(2):
                        for kt in range(KT):
                            nc.tensor.matmul(
                                ps[:, qc * 512:(qc + 1) * 512],
                                w_sb[:, kt, m * P:(m + 1) * P],
                                h1T[:, kt, qc * 512:(qc + 1) * 512],
                                start=(kt == 0), stop=(kt == KT - 1))
                    nc.vector.tensor_copy(qT[:, m, :], ps[:])

            # h1T freed here.

            # ---------------- cross K2/V2 (early, PE filler) ----------------
            k2T = vp.tile([P, KT, J], bf16, tag="k2T")
            w_sb = wbig.tile([P, KT, INNER], bf16, tag="w")
            nc.sync.dma_start(w_sb[:, 0:CKT, :], wk2_d[:])
            for m in range(KT):
                ps = ps_fill.tile([P, 512], f32, tag="fill")
                for kt in range(CKT):
                    nc.tensor.matmul(ps[:, 0:J], w_sb[:, kt, m * P:(m + 1) * P],
                                     ctxT_sb[:, kt, :],
                                     start=(kt == 0), stop=(kt == CKT - 1))
                nc.vector.tensor_copy(k2T[:, m, :], ps[:, 0:J])

            v2_sb = vp.tile([P, J // P, H, DH + 1], bf16, tag="v2")
            nc.vector.memset(v2_sb[:], 1.0)
            w_sb = wbig.tile([P, KT, INNER], bf16, tag="w")
            nc.sync.dma_start(w_sb[:, 0:CKT, :], wv2_d[:])
            for tt in range(J // P):
                for nc2 in range(2):
                    ps = ps_fill.tile([P, 512], f32, tag="fill")
                    for kt in range(CKT):
                        nc.tensor.matmul(
                            ps[:],
                            ctxT_sb[:, kt, tt * P:(tt + 1) * P],
                            w_sb[:, kt, nc2 * 512:(nc2 + 1) * 512],
                            start=(kt == 0), stop=(kt == CKT - 1))
                    nc.vector.tensor_copy(
                        v2_sb[:, tt, nc2 * 8:(nc2 + 1) * 8, 0:DH],
                        ps[:].rearrange("p (hh r) -> p hh r", r=DH))

            # ---------------- attention core ----------------
            with tc.tile_pool(name="expp", bufs=2) as expp, \
                 tc.tile_pool(name="atp", bufs=1) as atp, \
                 tc.tile_pool(name="kcp", bufs=2) as kcp:

                def attn_qc(qc, get_k, v_t, qT_t, nkt, out_T):
                    """All heads of one query-chunk. Separate PSUM rings per
                    role let the tile scheduler float independent dense work
                    into the PE stalls left by the scalar-engine softmax."""
                    qs = slice(qc * 512, (qc + 1) * 512)
                    for h in range(H):
                        hp = (h % 2) * 64
                        m2 = h // 2
                        kap = get_k(h)
                        ex = expp.tile([P, 16, 512], bf16, tag="ex")
                        for kt2 in range((nkt + 1) // 2):
                            ps = ps_big.tile([P, 1024], f32, tag="big")
                            nkk = min(2, nkt)
                            for u in range(nkk):
                                kt = kt2 * 2 + u
                                nc.tensor.matmul(
                                    ps[:, u * 512:(u + 1) * 512],
                                    kap[hp:hp + 64, kt * P:(kt + 1) * P],
                                    qT_t[hp:hp + 64, m2, qs],
                                    start=True, stop=True)
                            nc.scalar.activation(
                                ex[:, kt2 * 2:kt2 * 2 + nkk, :]
                                .rearrange("p a b -> p (a b)"),
                                ps[:, 0:nkk * 512], AF.Exp)
                        pav = ps_av.tile([P, 512], f32, tag="av")
                        for kt in range(nkt):
                            nc.tensor.matmul(
                                pav[0:65], v_t[:, kt, h, :], ex[:, kt, :],
                                start=(kt == 0), stop=(kt == nkt - 1))
                        den = small.tile([1, 512], f32, tag="den")
                        nc.vector.tensor_copy(den[:], pav[64:65, :])
                        rec32 = small.tile([1, 512], f32, tag="den",
                                           name="rec32")
                        nc.vector.reciprocal_approx_fast(rec32[:], den[:])
                        bcs = small.tile([64, 512], f32, tag="bcs")
                        nc.gpsimd.partition_broadcast(bcs[:], rec32[:])
                        if hp == 0:
                            nc.vector.tensor_tensor(out_T[0:64, m2, qs],
                                                    pav[0:64], bcs[:], ALU.mult)
                        else:
                            tmp = small.tile([64, 512], bf16, tag="todd")
                            nc.vector.tensor_tensor(tmp[:], pav[0:64],
                                                    bcs[:], ALU.mult)
                            nc.sync.dma_start(out_T[64:128, m2, qs], tmp[:])

                def out_proj(attn_T, w_t, brow_i, resid_src, out_dram, tt,
                             ln_grp, h_dst):
                    """o-proj + bias + residual for token tile tt; LN into
                    h_dst."""
                    xot = stg.tile([P, D], f32, tag="ostage")
                    for dc in range(2):
                        dsl = slice(dc * 512, (dc + 1) * 512)
                        ps = ps_fill.tile([P, 512], f32, tag="fill")
                        for m in range(KT):
                            nc.tensor.matmul(ps[:],
                                             attn_T[:, m, tt * P:(tt + 1) * P],
                                             w_t[:, m, dsl],
                                             start=(m == 0), stop=False)
                        nc.tensor.matmul(
                            ps[:], ones[0:1, :],
                            brow_sb[0:1, brow_i * D + dc * 512:
                                    brow_i * D + (dc + 1) * 512],
                            start=False, stop=True)
                        rt = xpool.tile([P, 512], f32, tag="x")
                        nc.sync.dma_start(rt[:], resid_src[tt, :, dsl])
                        nc.vector.tensor_tensor(xot[:, dsl], ps[:], rt[:],
                                                ALU.add)
                    nc.sync.dma_start(out_dram[tt], xot[:])
                    layernorm_tile(xot, ln_grp, h_dst, tt * P)

                # -------- self-attention + o1 + LN2, qc-interleaved --------
                attn1T = atp.tile([P, KT, N_OWN], bf16, tag="attnT")
                h2T = hTp.tile([P, KT, N_OWN], bf16, tag="hT")
                wo1_sb = wbig.tile([P, KT, INNER], bf16, tag="w")
                nc.sync.dma_start(wo1_sb[:], wo1_d[:])

                _kc = {}

                def get_k_self(h):
                    m2 = h // 2
                    if _kc.get("m2") != m2:
                        kth = kcp.tile([P, N], bf16, tag="kth")
                        nc.sync.dma_start(kth[:], kT_dram[m2])
                        _kc["m2"] = m2
                        _kc["t"] = kth
                    return _kc["t"]

                for qc in range(2):
                    _kc.clear()
                    attn_qc(qc, get_k_self, v_sb, qT, TT_FULL, attn1T)
                    for tt in range(qc * 4, qc * 4 + 4):
                        out_proj(attn1T, wo1_sb, 0, xf_d, x1_dram, tt, 1, h2T)
                    if qc == 0:
                        # prefetch q2 weights mid-attention (ring slot of wv2)
                        wq2_sb = wbig.tile([P, KT, INNER], bf16, tag="w")
                        nc.sync.dma_start(wq2_sb[:], wq2_d[:])

                # -------- q2 projection --------
                q2T = qTp.tile([P, KT, N_OWN], bf16, tag="qT")
                for m in range(KT):
                    for qc in range(2):
                        ps = ps_fill.tile([P, 512], f32, tag="fill")
                        for kt in range(KT):
                            nc.tensor.matmul(
                                ps[:],
                                wq2_sb[:, kt, m * P:(m + 1) * P],
                                h2T[:, kt, qc * 512:(qc + 1) * 512],
                                start=(kt == 0), stop=(kt == KT - 1))
                        nc.vector.tensor_copy(
                            q2T[:, m, qc * 512:(qc + 1) * 512], ps[:])

                # -------- cross-attention + o2 + LN3 --------
                attn2T = atp.tile([P, KT, N_OWN], bf16, tag="attnT")
                h3T = hTp.tile([P, KT, N_OWN], bf16, tag="hT")
                wo2_sb = wbig.tile([P, KT, INNER], bf16, tag="w")
                nc.sync.dma_start(wo2_sb[:], wo2_d[:])

                def get_k_cross(h):
                    return k2T[:, h // 2, :]

                for qc in range(2):
                    attn_qc(qc, get_k_cross, v2_sb, q2T, J // P, attn2T)
                    for tt in range(qc * 4, qc * 4 + 4):
                        out_proj(attn2T, wo2_sb, 1, x1_dram, x2_dram, tt, 2, h3T)

        # ---------------- Phase 6: GEGLU FF ----------------
        with tc.tile_pool(name="gp", bufs=1) as gp, \
             tc.tile_pool(name="wf2p", bufs=2) as wf2p:
            g_sb = gp.tile([P, 32, N_OWN], bf16, tag="g")
            for fc in range(32):
                wa = wsm.tile([P, KT, P], bf16, tag="wstream")
                nc.sync.dma_start(wa[:], wf1_d[:, :, fc * P:(fc + 1) * P])
                wg = wsm.tile([P, KT, P], bf16, tag="wstream")
                nc.sync.dma_start(wg[:], wf1_d[:, :, FF + fc * P:FF + (fc + 1) * P])
                for qc in range(2):
                    qs = slice(qc * 512, (qc + 1) * 512)
                    ps = ps_big.tile([P, 1024], f32, tag="big")
                    for kt in range(KT):
                        nc.tensor.matmul(ps[:, 0:512], wa[:, kt, :], h3T[:, kt, qs],
                                         start=(kt == 0), stop=(kt == KT - 1))
                    for kt in range(KT):
                        nc.tensor.matmul(ps[:, 512:1024], wg[:, kt, :],
                                         h3T[:, kt, qs],
                                         start=(kt == 0), stop=(kt == KT - 1))
                    gt = small.tile([P, 512], bf16, tag="gt")
                    nc.scalar.activation(gt[:], ps[:, 512:1024], AF.Gelu,
                                         bias=fb1_sb[:, 32 + fc:32 + fc + 1])
                    nc.vector.scalar_tensor_tensor(
                        g_sb[:, fc, qs], ps[:, 0:512], fb1_sb[:, fc:fc + 1],
                        gt[:], ALU.add, ALU.mult)

            for dc in range(4):
                dsl = slice(dc * 256, (dc + 1) * 256)
                wf2t = wf2p.tile([P, 32, 256], bf16, tag="wf2")
                nc.sync.dma_start(wf2t[:], wf2_d[:, :, dsl])
                for tt in range(TT_OWN):
                    ps = ps_fill.tile([P, 512], f32, tag="fill")
                    for j in range(32):
                        nc.tensor.matmul(ps[:, 0:256],
                                         g_sb[:, j, tt * P:(tt + 1) * P],
                                         wf2t[:, j, :],
                                         start=(j == 0), stop=False)
                    nc.tensor.matmul(ps[:, 0:256], ones[0:1, :],
                                     brow_sb[0:1, 2 * D + dc * 256:
                                             2 * D + (dc + 1) * 256],
                                     start=False, stop=True)
                    rt = xpool.tile([P, 512], f32, tag="x")
                    nc.sync.dma_start(rt[:, 0:256], x2_dram[tt, :, dsl])
                    yt = stg.tile([P, D], f32, tag="ostage")
                    nc.vector.tensor_tensor(yt[:, 0:256], ps[:, 0:256],
                                            rt[:, 0:256], ALU.add)
                    nc.sync.dma_start(y_d[tt, :, dsl], yt[:, 0:256])

    nc.compile()
    return nc


def _rearr_w(w, kt):
    return np.ascontiguousarray(
        w.reshape(kt, P, -1).transpose(1, 0, 2)).astype(BF16)


def _shard_inputs(inputs):
    f = {k: np.asarray(v, dtype=np.float32) for k, v in inputs.items()}
    shared = {
        "nw": _rearr_w(np.concatenate([f["n1_w"], f["n2_w"], f["n3_w"]], axis=1), KT),
        "nbc": np.ascontiguousarray(
            np.concatenate([f["n1_b"], f["n2_b"], f["n3_b"]])
            .reshape(3, 16, P).transpose(2, 0, 1).reshape(P, 48)),
        "wq1": _rearr_w(f["q1"] * SC, KT), "wk1": _rearr_w(f["k1"], KT),
        "wv1": _rearr_w(f["v1"], KT), "wo1": _rearr_w(f["o1_w"], KT),
        "wq2": _rearr_w(f["q2"] * SC, KT), "wk2": _rearr_w(f["k2"], CKT),
        "wv2": _rearr_w(f["v2"], CKT), "wo2": _rearr_w(f["o2_w"], KT),
        "brow": np.ascontiguousarray(
            np.concatenate([f["o1_b"], f["o2_b"], f["ff_b2"]])
            .reshape(1, 3 * D)).astype(BF16),
        "fb1c": np.ascontiguousarray(f["ff_b1"].reshape(64, P).T),
        "wf1": _rearr_w(f["ff_w1"], KT),
        "wf2": _rearr_w(f["ff_w2"], FF // P),
    }
    in_maps = []
    for core in range(8):
        b, half = core // 2, core % 2
        own = f["x"][b, half * N_OWN:(half + 1) * N_OWN]
        oth = f["x"][b, (1 - half) * N_OWN:(2 - half) * N_OWN]
        m = dict(shared)
        m["xf"] = np.ascontiguousarray(own.reshape(TT_OWN, P, D))
        m["xbf"] = np.ascontiguousarray(
            np.concatenate([own, oth]).reshape(TT_FULL, P, D)).astype(BF16)
        m["tT"] = np.ascontiguousarray(f["t"][b, 0].reshape(KT, P).T).astype(BF16)
        m["ctxT"] = np.ascontiguousarray(
            f["context"][b].T.reshape(CKT, P, J).transpose(1, 0, 2)).astype(BF16)
        in_maps.append(m)
    return in_maps


def kernel(**inputs):
    from concourse.bass_utils import run_bass_kernel_spmd
    if "nc" not in _CACHE:
        _CACHE["nc"] = _build_program()
    nc = _CACHE["nc"]
    in_maps = _shard_inputs(inputs)
    res = run_bass_kernel_spmd(nc, in_maps, core_ids=list(range(8)))
    out = np.empty((B, N, D), dtype=np.float32)
    for core in range(8):
        b, half = core // 2, core % 2
        out[b, half * N_OWN:(half + 1) * N_OWN] = \
            res.results[core]["y"].reshape(N_OWN, D)
    return out


# revision 25
# speedup vs baseline: 1.1463x; 1.1463x over previous
"""Trainium2 Bass kernel for nn_BasicTransformerBlock_35304631173827.

Sharding: 8 cores = 4 samples x 2 sequence halves. Each core computes its
1024-token half of one sample fully locally (self-attention K/V recomputed
over the full 2048-token sample -> zero collectives). bf16 matmuls with
fp32 PSUM accumulation; LayerNorm stats, softmax and residuals in fp32.

v2 restructure vs baseline:
- h1T kept SBUF-resident; dense projections use paired 512-col PSUM chains.
- Attention: every head's AV matmul is M=65 with a ones-column in V so the
  softmax denominator falls out of the same accumulation chain (no separate
  M=1 denominator matmuls); reciprocal_approx_fast for 1/den; odd heads'
  outputs shifted to partitions 64:128 via a small SBUF->SBUF DMA.
- qc-outer / head-inner loop with o-proj + LN interleaved to keep the PE
  array busy (p-state ramp) while the scalar engine chews softmax exps.
- FF: PSUM-accumulated FF2 (full K=4096 contraction in one chain), fused
  (a+b1)*gelu(gate+b1') via scalar_tensor_tensor, output biases folded into
  the matmul chains as K=1 ones-row accumulation steps.
"""

import numpy as np
import ml_dtypes

BF16 = ml_dtypes.bfloat16

B, N, D = 4, 2048, 1024
J, CD = 256, 768
H, DH = 16, 64
INNER = 1024
FF = 4096
P = 128
KT = D // P            # 8
CKT = CD // P          # 6
TT_FULL = N // P       # 16
N_OWN = N // 2
TT_OWN = N_OWN // P    # 8
EPS = 1e-5
SC = DH ** -0.5

_CACHE = {}


def _build_program():
    import concourse.tile as tile
    from concourse import mybir, bacc
    from concourse.masks import make_identity
    from contextlib import ExitStack

    f32 = mybir.dt.float32
    bf16 = mybir.dt.bfloat16
    AF = mybir.ActivationFunctionType
    ALU = mybir.AluOpType

    nc = bacc.Bacc(None, target_bir_lowering=False)

    xf_d = nc.dram_tensor("xf", [TT_OWN, P, D], f32, kind="ExternalInput")
    xbf_d = nc.dram_tensor("xbf", [TT_FULL, P, D], bf16, kind="ExternalInput")
    tT_d = nc.dram_tensor("tT", [P, KT], bf16, kind="ExternalInput")
    nw_d = nc.dram_tensor("nw", [P, KT, 6 * D], bf16, kind="ExternalInput")
    nbc_d = nc.dram_tensor("nbc", [P, 48], f32, kind="ExternalInput")
    wq1_d = nc.dram_tensor("wq1", [P, KT, INNER], bf16, kind="ExternalInput")
    wk1_d = nc.dram_tensor("wk1", [P, KT, INNER], bf16, kind="ExternalInput")
    wv1_d = nc.dram_tensor("wv1", [P, KT, INNER], bf16, kind="ExternalInput")
    wo1_d = nc.dram_tensor("wo1", [P, KT, D], bf16, kind="ExternalInput")
    wq2_d = nc.dram_tensor("wq2", [P, KT, INNER], bf16, kind="ExternalInput")
    wk2_d = nc.dram_tensor("wk2", [P, CKT, INNER], bf16, kind="ExternalInput")
    wv2_d = nc.dram_tensor("wv2", [P, CKT, INNER], bf16, kind="ExternalInput")
    wo2_d = nc.dram_tensor("wo2", [P, KT, D], bf16, kind="ExternalInput")
    ctxT_d = nc.dram_tensor("ctxT", [P, CKT, J], bf16, kind="ExternalInput")
    brow_d = nc.dram_tensor("brow", [1, 3 * D], bf16, kind="ExternalInput")
    fb1_d = nc.dram_tensor("fb1c", [P, 64], f32, kind="ExternalInput")
    wf1_d = nc.dram_tensor("wf1", [P, KT, 2 * FF], bf16, kind="ExternalInput")
    wf2_d = nc.dram_tensor("wf2", [P, FF // P, D], bf16, kind="ExternalInput")
    y_d = nc.dram_tensor("y", [TT_OWN, P, D], f32, kind="ExternalOutput")

    # DRAM scratch
    kT_dram = nc.dram_tensor("scr_kT", [KT, P, N], bf16, kind="Internal")
    x1_dram = nc.dram_tensor("scr_x1", [TT_OWN, P, D], f32, kind="Internal")
    x2_dram = nc.dram_tensor("scr_x2", [TT_OWN, P, D], f32, kind="Internal")

    with tile.TileContext(nc) as tc, ExitStack() as es:
        konst = es.enter_context(tc.tile_pool(name="konst", bufs=1))
        xpool = es.enter_context(tc.tile_pool(name="xpool", bufs=2))
        stats = es.enter_context(tc.tile_pool(name="stats", bufs=3))
        small = es.enter_context(tc.tile_pool(name="small", bufs=2))
        wsm = es.enter_context(tc.tile_pool(name="wsm", bufs=3))
        wbig = es.enter_context(tc.tile_pool(name="wbig", bufs=2))
        stg = es.enter_context(tc.tile_pool(name="stg", bufs=2))
        hTp = es.enter_context(tc.tile_pool(name="hTp", bufs=1))
        ps_big = es.enter_context(tc.tile_pool(name="ps_big", bufs=2, space="PSUM"))
        ps_av = es.enter_context(tc.tile_pool(name="ps_av", bufs=2, space="PSUM"))
        ps_bc = es.enter_context(tc.tile_pool(name="ps_bc", bufs=2, space="PSUM"))

        # ---------------- constants ----------------
        ident = konst.tile([P, P], bf16)
        make_identity(nc, ident)
        ones = konst.tile([1, P], bf16)
        nc.vector.memset(ones[:], 1.0)
        eps_t = konst.tile([P, 1], f32)
        nc.vector.memset(eps_t[:], EPS)
        tT_sb = konst.tile([P, KT], bf16)
        nc.sync.dma_start(tT_sb[:], tT_d[:])
        nbc_sb = konst.tile([P, 48], f32)
        nc.sync.dma_start(nbc_sb[:], nbc_d[:])
        fb1_sb = konst.tile([P, 64], f32)
        nc.sync.dma_start(fb1_sb[:], fb1_d[:])
        brow_sb = konst.tile([1, 3 * D], bf16)
        nc.sync.dma_start(brow_sb[:], brow_d[:])
        cols = konst.tile([P, 48], f32)

        # ---------------- Phase 0: AdaLN embeddings ----------------
        # emb^T chunks: cols[:, cc] = (t @ nW)[cc*128 : (cc+1)*128]
        # Only the n1 group is computed up front (it gates LN1); n2/n3 are
        # deferred past the K projection to keep the cold-start DMA window
        # small.
        def emb_group(g):
            for c in range(16):
                cc = g * 16 + c
                nwt = wsm.tile([P, KT, P], bf16, tag="wstream")
                nc.sync.dma_start(nwt[:], nw_d[:, :, cc * P:(cc + 1) * P])
                ps = ps_av.tile([P, 512], f32, tag="av")
                for kt in range(KT):
                    nc.tensor.matmul(ps[:, 0:1], nwt[:, kt, :], tT_sb[:, kt:kt + 1],
                                     start=(kt == 0), stop=(kt == KT - 1))
                nc.vector.tensor_copy(cols[:, cc:cc + 1], ps[:, 0:1])
            sl = slice(g * 16, g * 16 + 16)
            nc.vector.tensor_add(cols[:, sl], cols[:, sl], nbc_sb[:, sl])
            nc.vector.tensor_scalar_add(cols[:, g * 16:g * 16 + 8],
                                        cols[:, g * 16:g * 16 + 8], 1.0)

        emb_group(0)

        def layernorm_tile(x_tile, n3, dst_sb, off):
            """LayerNorm + AdaLN affine on (P, D) tile -> transposed chunks
            written to dst_sb[:, c, off:off+128]."""
            bst = stats.tile([P, 2, 6], f32, tag="bnst")
            for g in range(2):
                nc.vector.bn_stats(bst[:, g, :], x_tile[:, g * 512:(g + 1) * 512])
            mv = stats.tile([P, 4], f32, tag="mv")
            nc.vector.bn_aggr(mv[:, 0:2], bst[:])
            nc.scalar.activation(mv[:, 2:3], mv[:, 1:2], AF.Sqrt, bias=eps_t[:])
            nc.vector.reciprocal(mv[:, 2:3], mv[:, 2:3])
            nc.vector.tensor_tensor(mv[:, 3:4], mv[:, 0:1], mv[:, 2:3], ALU.mult)
            nc.vector.tensor_scalar_mul(mv[:, 3:4], mv[:, 3:4], -1.0)
            xn = small.tile([P, D], bf16, tag="xn")
            nc.scalar.activation(xn[:], x_tile[:], AF.Identity,
                                 bias=mv[:, 3:4], scale=mv[:, 2:3])
            for c in range(KT):
                pt = ps_bc.tile([P, P], bf16, tag="bc", name="pt")
                nc.tensor.transpose(pt[:], xn[:, c * P:(c + 1) * P], ident[:])
                nc.vector.tensor_scalar(
                    dst_sb[:, c, off:off + P], pt[:],
                    cols[:, n3 * 16 + c:n3 * 16 + c + 1],
                    cols[:, n3 * 16 + 8 + c:n3 * 16 + 8 + c + 1],
                    ALU.mult, ALU.add)

        # outer-attention scope: qT/q2T, v tiles
        with tc.tile_pool(name="qTp", bufs=1) as qTp, \
             tc.tile_pool(name="vp", bufs=1) as vp:

            ctxT_sb = vp.tile([P, CKT, J], bf16, tag="ctx")
            nc.sync.dma_start(ctxT_sb[:], ctxT_d[:])

            # ---------------- Phase 1+2: LN1 -> h1T (SBUF); QKV ----------------
            # LN1 interleaved with V-projection in 4-tile token groups so the
            # PE array has matmul work while LN chains stream through DVE.
            with tc.tile_pool(name="h1p", bufs=1) as h1p, \
                 tc.tile_pool(name="xbp", bufs=3) as xbp:
                h1T = h1p.tile([P, KT, N], bf16, tag="h1T")
                v_sb = vp.tile([P, TT_FULL, H, DH + 1], bf16, tag="v1")
                nc.vector.memset(v_sb[:], 1.0)
                wv_sb = wbig.tile([P, KT, INNER], bf16, tag="w")
                nc.sync.dma_start(wv_sb[:], wv1_d[:])
                def v_group(grp):
                    for tt in range(grp * 4, grp * 4 + 4):
                        ps = ps_big.tile([P, 1024], f32, tag="big")
                        for nc2 in range(2):
                            for kt in range(KT):
                                nc.tensor.matmul(
                                    ps[:, nc2 * 512:(nc2 + 1) * 512],
                                    h1T[:, kt, tt * P:(tt + 1) * P],
                                    wv_sb[:, kt, nc2 * 512:(nc2 + 1) * 512],
                                    start=(kt == 0), stop=(kt == KT - 1))
                        nc.vector.tensor_copy(
                            v_sb[:, tt, :, 0:DH],
                            ps[:].rearrange("p (hh r) -> p hh r", r=DH))

                for grp in range(4):
                    for tt in range(grp * 4, grp * 4 + 4):
                        xt = xbp.tile([P, D], bf16, tag="xb")
                        nc.sync.dma_start(xt[:], xbf_d[tt])
                        layernorm_tile(xt, 0, h1T, tt * P)
                    if grp > 0:
                        v_group(grp - 1)
                v_group(3)

                # K projection (full sample) -> DRAM
                w_sb = wbig.tile([P, KT, INNER], bf16, tag="w")
                nc.sync.dma_start(w_sb[:], wk1_d[:])
                for m in range(KT):
                    for half in range(2):
                        ps = ps_big.tile([P, 1024], f32, tag="big")
                        for qc in range(2):
                            for kt in range(KT):
                                nc.tensor.matmul(
                                    ps[:, qc * 512:(qc + 1) * 512],
                                    w_sb[:, kt, m * P:(m + 1) * P],
                                    h1T[:, kt, half * 1024 + qc * 512:
                                        half * 1024 + (qc + 1) * 512],
                                    start=(kt == 0), stop=(kt == KT - 1))
                        kst = xbp.tile([P, 1024], bf16, tag="kst")
                        nc.vector.tensor_copy(kst[:], ps[:])
                        nc.sync.dma_start(
                            kT_dram[m, :, half * 1024:(half + 1) * 1024], kst[:])

                # deferred AdaLN embedding groups (needed from LN2 on)
                emb_group(1)
                emb_group(2)

                # Q projection (own half, softmax scale pre-folded into wq1)
                qT = qTp.tile([P, KT, N_OWN], bf16, tag="qT")
                w_sb = wbig.tile([P, KT, INNER], bf16, tag="w")
                nc.sync.dma_start(w_sb[:], wq1_d[:])
                for m in range(KT):
                    ps = ps_big.tile([P, 1024], f32, tag="big")
                    for qc in range(2):
                        for kt in range(KT):
                            nc.tensor.matmul(
                                ps[:, qc * 512:(qc + 1) * 512],
                                w_sb[:, kt, m * P:(m + 1) * P],
                                h1T[:, kt, qc * 512:(qc + 1) * 512],
                                start=(kt == 0), stop=(kt == KT - 1))
                    nc.vector.tensor_copy(qT[:, m, :], ps[:])

            # h1T freed here.

            # ---------------- cross K2/V2 (early, PE filler) ----------------
            k2T = vp.tile([P, KT, J], bf16, tag="k2T")
            w_sb = wbig.tile([P, KT, INNER], bf16, tag="w")
            nc.sync.dma_start(w_sb[:, 0:CKT, :], wk2_d[:])
            for m in range(KT):
                ps = ps_av.tile([P, 512], f32, tag="av")
                for kt in range(CKT):
                    nc.tensor.matmul(ps[:, 0:J], w_sb[:, kt, m * P:(m + 1) * P],
                                     ctxT_sb[:, kt, :],
                                     start=(kt == 0), stop=(kt == CKT - 1))
                nc.vector.tensor_copy(k2T[:, m, :], ps[:, 0:J])

            v2_sb = vp.tile([P, J // P, H, DH + 1], bf16, tag="v2")
            nc.vector.memset(v2_sb[:], 1.0)
            w_sb = wbig.tile([P, KT, INNER], bf16, tag="w")
            nc.sync.dma_start(w_sb[:, 0:CKT, :], wv2_d[:])
            for tt in range(J // P):
                ps = ps_big.tile([P, 1024], f32, tag="big")
                for nc2 in range(2):
                    for kt in range(CKT):
                        nc.tensor.matmul(
                            ps[:, nc2 * 512:(nc2 + 1) * 512],
                            ctxT_sb[:, kt, tt * P:(tt + 1) * P],
                            w_sb[:, kt, nc2 * 512:(nc2 + 1) * 512],
                            start=(kt == 0), stop=(kt == CKT - 1))
                nc.vector.tensor_copy(
                    v2_sb[:, tt, :, 0:DH],
                    ps[:].rearrange("p (hh r) -> p hh r", r=DH))

            # ---------------- attention core ----------------
            with tc.tile_pool(name="expp", bufs=3) as expp, \
                 tc.tile_pool(name="atp", bufs=1) as atp, \
                 tc.tile_pool(name="kcp", bufs=2) as kcp:

                def attn_qc(qc, get_k, v_t, qT_t, nkt, out_T):
                    """All heads of one query-chunk of attention."""
                    qs = slice(qc * 512, (qc + 1) * 512)
                    for h in range(H):
                        hp = (h % 2) * 64
                        m2 = h // 2
                        kap = get_k(h)
                        exs = []
                        for half in range((nkt + 7) // 8):
                            ex = expp.tile([P, 8, 512], bf16, tag="ex")
                            exs.append(ex)
                        for kt2 in range((nkt + 1) // 2):
                            ps = ps_big.tile([P, 1024], f32, tag="big")
                            nkk = min(2, nkt)
                            for u in range(nkk):
                                kt = kt2 * 2 + u
                                nc.tensor.matmul(
                                    ps[:, u * 512:(u + 1) * 512],
                                    kap[hp:hp + 64, kt * P:(kt + 1) * P],
                                    qT_t[hp:hp + 64, m2, qs],
                                    start=True, stop=True)
                            ex = exs[kt2 // 4]
                            lo = (kt2 % 4) * 2
                            nc.scalar.activation(
                                ex[:, lo:lo + nkk, :]
                                .rearrange("p a b -> p (a b)"),
                                ps[:, 0:nkk * 512], AF.Exp)
                        pav = ps_av.tile([P, 512], f32, tag="av")
                        for kt in range(nkt):
                            nc.tensor.matmul(
                                pav[0:65], v_t[:, kt, h, :],
                                exs[kt // 8][:, kt % 8, :],
                                start=(kt == 0), stop=(kt == nkt - 1))
                        den = small.tile([1, 512], f32, tag="den")
                        nc.vector.tensor_copy(den[:], pav[64:65, :])
                        rec32 = small.tile([1, 512], f32, tag="den",
                                           name="rec32")
                        nc.vector.reciprocal_approx_fast(rec32[:], den[:])
                        bcs = small.tile([64, 512], f32, tag="bcs")
                        nc.gpsimd.partition_broadcast(bcs[:], rec32[:])
                        if hp == 0:
                            nc.vector.tensor_tensor(out_T[0:64, m2, qs],
                                                    pav[0:64], bcs[:], ALU.mult)
                        else:
                            tmp = small.tile([64, 512], bf16, tag="todd")
                            nc.vector.tensor_tensor(tmp[:], pav[0:64],
                                                    bcs[:], ALU.mult)
                            nc.sync.dma_start(out_T[64:128, m2, qs], tmp[:])

                def out_proj(attn_T, w_t, brow_i, resid_src, out_dram, tt,
                             ln_grp, h_dst):
                    """o-proj + bias + residual for token tile tt; LN into
                    h_dst."""
                    ps = ps_big.tile([P, 1024], f32, tag="big")
                    for dc in range(2):
                        dsl = slice(dc * 512, (dc + 1) * 512)
                        for m in range(KT):
                            nc.tensor.matmul(ps[:, dsl],
                                             attn_T[:, m, tt * P:(tt + 1) * P],
                                             w_t[:, m, dsl],
                                             start=(m == 0), stop=False)
                        nc.tensor.matmul(
                            ps[:, dsl], ones[0:1, :],
                            brow_sb[0:1, brow_i * D + dc * 512:
                                    brow_i * D + (dc + 1) * 512],
                            start=False, stop=True)
                    xot = stg.tile([P, D], f32, tag="ostage")
                    for dc in range(2):
                        dsl = slice(dc * 512, (dc + 1) * 512)
                        rt = xpool.tile([P, 512], f32, tag="x")
                        nc.sync.dma_start(rt[:], resid_src[tt, :, dsl])
                        nc.vector.tensor_tensor(xot[:, dsl], ps[:, dsl], rt[:],
                                                ALU.add)
                    nc.sync.dma_start(out_dram[tt], xot[:])
                    layernorm_tile(xot, ln_grp, h_dst, tt * P)

                # -------- self-attention + o1 + LN2, qc-interleaved --------
                attn1T = atp.tile([P, KT, N_OWN], bf16, tag="attnT")
                h2T = hTp.tile([P, KT, N_OWN], bf16, tag="hT")
                wo1_sb = wbig.tile([P, KT, INNER], bf16, tag="w")
                nc.sync.dma_start(wo1_sb[:], wo1_d[:])

                _kc = {}

                def get_k_self(h):
                    m2 = h // 2
                    if _kc.get("m2") != m2:
                        kth = kcp.tile([P, N], bf16, tag="kth")
                        nc.sync.dma_start(kth[:], kT_dram[m2])
                        _kc["m2"] = m2
                        _kc["t"] = kth
                    return _kc["t"]

                for qc in range(2):
                    _kc.clear()
                    attn_qc(qc, get_k_self, v_sb, qT, TT_FULL, attn1T)
                    for tt in range(qc * 4, qc * 4 + 4):
                        out_proj(attn1T, wo1_sb, 0, xf_d, x1_dram, tt, 1, h2T)
                    if qc == 0:
                        # prefetch q2 weights mid-attention (ring slot of wv2)
                        wq2_sb = wbig.tile([P, KT, INNER], bf16, tag="w")
                        nc.sync.dma_start(wq2_sb[:], wq2_d[:])

                # -------- q2 projection --------
                q2T = qTp.tile([P, KT, N_OWN], bf16, tag="qT")
                for m in range(KT):
                    ps = ps_big.tile([P, 1024], f32, tag="big")
                    for qc in range(2):
                        for kt in range(KT):
                            nc.tensor.matmul(
                                ps[:, qc * 512:(qc + 1) * 512],
                                wq2_sb[:, kt, m * P:(m + 1) * P],
                                h2T[:, kt, qc * 512:(qc + 1) * 512],
                                start=(kt == 0), stop=(kt == KT - 1))
                    nc.vector.tensor_copy(q2T[:, m, :], ps[:])

                # -------- cross-attention + o2 + LN3 --------
                attn2T = atp.tile([P, KT, N_OWN], bf16, tag="attnT")
                h3T = hTp.tile([P, KT, N_OWN], bf16, tag="hT")
                wo2_sb = wbig.tile([P, KT, INNER], bf16, tag="w")
                nc.sync.dma_start(wo2_sb[:], wo2_d[:])

                def get_k_cross(h):
                    return k2T[:, h // 2, :]

                for qc in range(2):
                    attn_qc(qc, get_k_cross, v2_sb, q2T, J // P, attn2T)
                    for tt in range(qc * 4, qc * 4 + 4):
                        out_proj(attn2T, wo2_sb, 1, x1_dram, x2_dram, tt, 2, h3T)

        # ---------------- Phase 6: GEGLU FF ----------------
        with tc.tile_pool(name="gp", bufs=1) as gp, \
             tc.tile_pool(name="wf2p", bufs=2) as wf2p:
            g_sb = gp.tile([P, 32, N_OWN], bf16, tag="g")
            for fc in range(32):
                wa = wsm.tile([P, KT, P], bf16, tag="wstream")
                nc.sync.dma_start(wa[:], wf1_d[:, :, fc * P:(fc + 1) * P])
                wg = wsm.tile([P, KT, P], bf16, tag="wstream")
                nc.sync.dma_start(wg[:], wf1_d[:, :, FF + fc * P:FF + (fc + 1) * P])
                for qc in range(2):
                    qs = slice(qc * 512, (qc + 1) * 512)
                    ps = ps_big.tile([P, 1024], f32, tag="big")
                    for kt in range(KT):
                        nc.tensor.matmul(ps[:, 0:512], wa[:, kt, :], h3T[:, kt, qs],
                                         start=(kt == 0), stop=(kt == KT - 1))
                    for kt in range(KT):
                        nc.tensor.matmul(ps[:, 512:1024], wg[:, kt, :],
                                         h3T[:, kt, qs],
                                         start=(kt == 0), stop=(kt == KT - 1))
                    gt = small.tile([P, 512], bf16, tag="gt")
                    nc.scalar.activation(gt[:], ps[:, 512:1024], AF.Gelu,
                                         bias=fb1_sb[:, 32 + fc:32 + fc + 1])
                    nc.vector.scalar_tensor_tensor(
                        g_sb[:, fc, qs], ps[:, 0:512], fb1_sb[:, fc:fc + 1],
                        gt[:], ALU.add, ALU.mult)

            for dc in range(4):
                dsl = slice(dc * 256, (dc + 1) * 256)
                wf2t = wf2p.tile([P, 32, 256], bf16, tag="wf2")
                nc.sync.dma_start(wf2t[:], wf2_d[:, :, dsl])
                for tt in range(TT_OWN):
                    ps = ps_av.tile([P, 512], f32, tag="av")
                    for j in range(32):
                        nc.tensor.matmul(ps[:, 0:256],
                                         g_sb[:, j, tt * P:(tt + 1) * P],
                                         wf2t[:, j, :],
                                         start=(j == 0), stop=False)
                    nc.tensor.matmul(ps[:, 0:256], ones[0:1, :],
                                     brow_sb[0:1, 2 * D + dc * 256:
                                             2 * D + (dc + 1) * 256],
                                     start=False, stop=True)
                    rt = xpool.tile([P, 512], f32, tag="x")
                    nc.sync.dma_start(rt[:, 0:256], x2_dram[tt, :, dsl])
                    yt = stg.tile([P, D], f32, tag="ostage")
                    nc.vector.tensor_tensor(yt[:, 0:256], ps[:, 0:256],
                                            rt[:, 0:256], ALU.add)
                    nc.sync.dma_start(y_d[tt, :, dsl], yt[:, 0:256])

    nc.compile()
    return nc


def _rearr_w(w, kt):
    return np.ascontiguousarray(
        w.reshape(kt, P, -1).transpose(1, 0, 2)).astype(BF16)


def _shard_inputs(inputs):
    f = {k: np.asarray(v, dtype=np.float32) for k, v in inputs.items()}
    shared = {
        "nw": _rearr_w(np.concatenate([f["n1_w"], f["n2_w"], f["n3_w"]], axis=1), KT),
        "nbc": np.ascontiguousarray(
            np.concatenate([f["n1_b"], f["n2_b"], f["n3_b"]])
            .reshape(3, 16, P).transpose(2, 0, 1).reshape(P, 48)),
        "wq1": _rearr_w(f["q1"] * SC, KT), "wk1": _rearr_w(f["k1"], KT),
        "wv1": _rearr_w(f["v1"], KT), "wo1": _rearr_w(f["o1_w"], KT),
        "wq2": _rearr_w(f["q2"] * SC, KT), "wk2": _rearr_w(f["k2"], CKT),
        "wv2": _rearr_w(f["v2"], CKT), "wo2": _rearr_w(f["o2_w"], KT),
        "brow": np.ascontiguousarray(
            np.concatenate([f["o1_b"], f["o2_b"], f["ff_b2"]])
            .reshape(1, 3 * D)).astype(BF16),
        "fb1c": np.ascontiguousarray(f["ff_b1"].reshape(64, P).T),
        "wf1": _rearr_w(f["ff_w1"], KT),
        "wf2": _rearr_w(f["ff_w2"], FF // P),
    }
    in_maps = []
    for core in range(8):
        b, half = core // 2, core % 2
        own = f["x"][b, half * N_OWN:(half + 1) * N_OWN]
        oth = f["x"][b, (1 - half) * N_OWN:(2 - half) * N_OWN]
        m = dict(shared)
        m["xf"] = np.ascontiguousarray(own.reshape(TT_OWN, P, D))
        m["xbf"] = np.ascontiguousarray(
            np.concatenate([own, oth]).reshape(TT_FULL, P, D)).astype(BF16)
        m["tT"] = np.ascontiguousarray(f["t"][b, 0].reshape(KT, P).T).astype(BF16)
        m["ctxT"] = np.ascontiguousarray(
            f["context"][b].T.reshape(CKT, P, J).transpose(1, 0, 2)).astype(BF16)
        in_maps.append(m)
    return in_maps


def kernel(**inputs):
    from concourse.bass_utils import run_bass_kernel_spmd
    if "nc" not in _CACHE:
        _CACHE["nc"] = _build_program()
    nc = _CACHE["nc"]
    in_maps = _shard_inputs(inputs)
    res = run_bass_kernel_spmd(nc, in_maps, core_ids=list(range(8)))
    out = np.empty((B, N, D), dtype=np.float32)
    for core in range(8):
        b, half = core // 2, core % 2
        out[b, half * N_OWN:(half + 1) * N_OWN] = \
            res.results[core]["y"].reshape(N_OWN, D)
    return out


# revision 26
# speedup vs baseline: 1.2156x; 1.0605x over previous
"""Trainium2 Bass kernel for nn_BasicTransformerBlock_35304631173827.

Sharding: 8 cores = 4 samples x 2 sequence halves. Each core computes its
1024-token half of one sample fully locally (self-attention K/V recomputed
over the full 2048-token sample -> zero collectives). bf16 matmuls with
fp32 PSUM accumulation; LayerNorm stats, softmax and residuals in fp32.

v2 restructure vs baseline:
- h1T kept SBUF-resident; dense projections use paired 512-col PSUM chains.
- Attention: every head's AV matmul is M=65 with a ones-column in V so the
  softmax denominator falls out of the same accumulation chain (no separate
  M=1 denominator matmuls); reciprocal_approx_fast for 1/den; odd heads'
  outputs shifted to partitions 64:128 via a small SBUF->SBUF DMA.
- qc-outer / head-inner loop with o-proj + LN interleaved to keep the PE
  array busy (p-state ramp) while the scalar engine chews softmax exps.
- FF: PSUM-accumulated FF2 (full K=4096 contraction in one chain), fused
  (a+b1)*gelu(gate+b1') via scalar_tensor_tensor, output biases folded into
  the matmul chains as K=1 ones-row accumulation steps.
"""

import numpy as np
import ml_dtypes

BF16 = ml_dtypes.bfloat16

B, N, D = 4, 2048, 1024
J, CD = 256, 768
H, DH = 16, 64
INNER = 1024
FF = 4096
P = 128
KT = D // P            # 8
CKT = CD // P          # 6
TT_FULL = N // P       # 16
N_OWN = N // 2
TT_OWN = N_OWN // P    # 8
EPS = 1e-5
SC = DH ** -0.5

_CACHE = {}


def _build_program():
    import concourse.tile as tile
    from concourse import mybir, bacc
    from concourse.masks import make_identity
    from contextlib import ExitStack

    f32 = mybir.dt.float32
    bf16 = mybir.dt.bfloat16
    AF = mybir.ActivationFunctionType
    ALU = mybir.AluOpType

    nc = bacc.Bacc(None, target_bir_lowering=False)

    xf_d = nc.dram_tensor("xf", [TT_OWN, P, D], f32, kind="ExternalInput")
    xbf_d = nc.dram_tensor("xbf", [TT_FULL, P, D], bf16, kind="ExternalInput")
    tT_d = nc.dram_tensor("tT", [P, KT], bf16, kind="ExternalInput")
    nw_d = nc.dram_tensor("nw", [P, KT, 6 * D], bf16, kind="ExternalInput")
    nbc_d = nc.dram_tensor("nbc", [P, 48], f32, kind="ExternalInput")
    wq1_d = nc.dram_tensor("wq1", [P, KT, INNER], bf16, kind="ExternalInput")
    wk1_d = nc.dram_tensor("wk1", [P, KT, INNER], bf16, kind="ExternalInput")
    wv1_d = nc.dram_tensor("wv1", [P, KT, INNER], bf16, kind="ExternalInput")
    wo1_d = nc.dram_tensor("wo1", [P, KT, D], bf16, kind="ExternalInput")
    wq2_d = nc.dram_tensor("wq2", [P, KT, INNER], bf16, kind="ExternalInput")
    wk2_d = nc.dram_tensor("wk2", [P, CKT, INNER], bf16, kind="ExternalInput")
    wv2_d = nc.dram_tensor("wv2", [P, CKT, INNER], bf16, kind="ExternalInput")
    wo2_d = nc.dram_tensor("wo2", [P, KT, D], bf16, kind="ExternalInput")
    ctxT_d = nc.dram_tensor("ctxT", [P, CKT, J], bf16, kind="ExternalInput")
    brow_d = nc.dram_tensor("brow", [1, 3 * D], bf16, kind="ExternalInput")
    fb1_d = nc.dram_tensor("fb1c", [P, 64], f32, kind="ExternalInput")
    wf1_d = nc.dram_tensor("wf1", [P, KT, 2 * FF], bf16, kind="ExternalInput")
    wf2_d = nc.dram_tensor("wf2", [P, FF // P, D], bf16, kind="ExternalInput")
    y_d = nc.dram_tensor("y", [TT_OWN, P, D], f32, kind="ExternalOutput")

    # DRAM scratch
    kT_dram = nc.dram_tensor("scr_kT", [KT, P, N], bf16, kind="Internal")
    x1_dram = nc.dram_tensor("scr_x1", [TT_OWN, P, D], f32, kind="Internal")
    x2_dram = nc.dram_tensor("scr_x2", [TT_OWN, P, D], f32, kind="Internal")

    with tile.TileContext(nc) as tc, ExitStack() as es:
        konst = es.enter_context(tc.tile_pool(name="konst", bufs=1))
        xpool = es.enter_context(tc.tile_pool(name="xpool", bufs=2))
        stats = es.enter_context(tc.tile_pool(name="stats", bufs=3))
        small = es.enter_context(tc.tile_pool(name="small", bufs=2))
        wsm = es.enter_context(tc.tile_pool(name="wsm", bufs=3))
        wbig = es.enter_context(tc.tile_pool(name="wbig", bufs=2))
        stg = es.enter_context(tc.tile_pool(name="stg", bufs=2))
        hTp = es.enter_context(tc.tile_pool(name="hTp", bufs=1))
        ps_big = es.enter_context(tc.tile_pool(name="ps_big", bufs=2, space="PSUM"))
        ps_av = es.enter_context(tc.tile_pool(name="ps_av", bufs=2, space="PSUM"))
        ps_bc = es.enter_context(tc.tile_pool(name="ps_bc", bufs=2, space="PSUM"))

        # ---------------- constants ----------------
        ident = konst.tile([P, P], bf16)
        make_identity(nc, ident)
        ones = konst.tile([1, P], bf16)
        nc.vector.memset(ones[:], 1.0)
        eps_t = konst.tile([P, 1], f32)
        nc.vector.memset(eps_t[:], EPS)
        tT_sb = konst.tile([P, KT], bf16)
        nc.sync.dma_start(tT_sb[:], tT_d[:])
        nbc_sb = konst.tile([P, 48], f32)
        nc.sync.dma_start(nbc_sb[:], nbc_d[:])
        fb1_sb = konst.tile([P, 64], f32)
        nc.sync.dma_start(fb1_sb[:], fb1_d[:])
        brow_sb = konst.tile([1, 3 * D], bf16)
        nc.sync.dma_start(brow_sb[:], brow_d[:])
        cols = konst.tile([P, 48], f32)

        # ---------------- Phase 0: AdaLN embeddings ----------------
        # emb^T chunks: cols[:, cc] = (t @ nW)[cc*128 : (cc+1)*128]
        # Only the n1 group is computed up front (it gates LN1); n2/n3 are
        # deferred past the K projection to keep the cold-start DMA window
        # small.
        def emb_group(g):
            for c in range(16):
                cc = g * 16 + c
                nwt = wsm.tile([P, KT, P], bf16, tag="wstream")
                nc.sync.dma_start(nwt[:], nw_d[:, :, cc * P:(cc + 1) * P])
                ps = ps_av.tile([P, 512], f32, tag="av")
                for kt in range(KT):
                    nc.tensor.matmul(ps[:, 0:1], nwt[:, kt, :], tT_sb[:, kt:kt + 1],
                                     start=(kt == 0), stop=(kt == KT - 1))
                nc.vector.tensor_copy(cols[:, cc:cc + 1], ps[:, 0:1])
            sl = slice(g * 16, g * 16 + 16)
            nc.vector.tensor_add(cols[:, sl], cols[:, sl], nbc_sb[:, sl])
            nc.vector.tensor_scalar_add(cols[:, g * 16:g * 16 + 8],
                                        cols[:, g * 16:g * 16 + 8], 1.0)

        emb_group(0)

        def layernorm_tile(x_tile, n3, dst_sb, off):
            """LayerNorm + AdaLN affine on (P, D) tile -> transposed chunks
            written to dst_sb[:, c, off:off+128]."""
            bst = stats.tile([P, 2, 6], f32, tag="bnst")
            for g in range(2):
                nc.vector.bn_stats(bst[:, g, :], x_tile[:, g * 512:(g + 1) * 512])
            mv = stats.tile([P, 4], f32, tag="mv")
            nc.vector.bn_aggr(mv[:, 0:2], bst[:])
            nc.scalar.activation(mv[:, 2:3], mv[:, 1:2], AF.Sqrt, bias=eps_t[:])
            nc.vector.reciprocal(mv[:, 2:3], mv[:, 2:3])
            nc.vector.tensor_tensor(mv[:, 3:4], mv[:, 0:1], mv[:, 2:3], ALU.mult)
            nc.vector.tensor_scalar_mul(mv[:, 3:4], mv[:, 3:4], -1.0)
            xn = small.tile([P, D], bf16, tag="xn")
            nc.scalar.activation(xn[:], x_tile[:], AF.Identity,
                                 bias=mv[:, 3:4], scale=mv[:, 2:3])
            for c in range(KT):
                pt = ps_bc.tile([P, P], bf16, tag="bc", name="pt")
                nc.tensor.transpose(pt[:], xn[:, c * P:(c + 1) * P], ident[:])
                nc.vector.tensor_scalar(
                    dst_sb[:, c, off:off + P], pt[:],
                    cols[:, n3 * 16 + c:n3 * 16 + c + 1],
                    cols[:, n3 * 16 + 8 + c:n3 * 16 + 8 + c + 1],
                    ALU.mult, ALU.add)

        # outer-attention scope: qT/q2T, v tiles
        with tc.tile_pool(name="qTp", bufs=1) as qTp, \
             tc.tile_pool(name="vp", bufs=1) as vp:

            ctxT_sb = vp.tile([P, CKT, J], bf16, tag="ctx")
            nc.sync.dma_start(ctxT_sb[:], ctxT_d[:])

            # ---------------- Phase 1+2: LN1 -> h1T (SBUF); QKV ----------------
            # LN1 interleaved with V-projection in 4-tile token groups so the
            # PE array has matmul work while LN chains stream through DVE.
            with tc.tile_pool(name="h1p", bufs=1) as h1p, \
                 tc.tile_pool(name="xbp", bufs=3) as xbp:
                h1T = h1p.tile([P, KT, N], bf16, tag="h1T")
                v_sb = vp.tile([P, TT_FULL, H, DH + 1], bf16, tag="v1")
                nc.vector.memset(v_sb[:], 1.0)
                wv_sb = wbig.tile([P, KT, INNER], bf16, tag="w")
                nc.sync.dma_start(wv_sb[:], wv1_d[:])
                def v_group(grp):
                    for tt in range(grp * 4, grp * 4 + 4):
                        ps = ps_big.tile([P, 1024], f32, tag="big")
                        for nc2 in range(2):
                            for kt in range(KT):
                                nc.tensor.matmul(
                                    ps[:, nc2 * 512:(nc2 + 1) * 512],
                                    h1T[:, kt, tt * P:(tt + 1) * P],
                                    wv_sb[:, kt, nc2 * 512:(nc2 + 1) * 512],
                                    start=(kt == 0), stop=(kt == KT - 1))
                        nc.vector.tensor_copy(
                            v_sb[:, tt, :, 0:DH],
                            ps[:].rearrange("p (hh r) -> p hh r", r=DH))

                for grp in range(4):
                    for tt in range(grp * 4, grp * 4 + 4):
                        xt = xbp.tile([P, D], bf16, tag="xb")
                        nc.sync.dma_start(xt[:], xbf_d[tt])
                        layernorm_tile(xt, 0, h1T, tt * P)
                    if grp > 0:
                        v_group(grp - 1)
                v_group(3)

                # K projection (full sample) -> DRAM
                w_sb = wbig.tile([P, KT, INNER], bf16, tag="w")
                nc.sync.dma_start(w_sb[:], wk1_d[:])
                for m in range(KT):
                    for half in range(2):
                        ps = ps_big.tile([P, 1024], f32, tag="big")
                        for qc in range(2):
                            for kt in range(KT):
                                nc.tensor.matmul(
                                    ps[:, qc * 512:(qc + 1) * 512],
                                    w_sb[:, kt, m * P:(m + 1) * P],
                                    h1T[:, kt, half * 1024 + qc * 512:
                                        half * 1024 + (qc + 1) * 512],
                                    start=(kt == 0), stop=(kt == KT - 1))
                        kst = xbp.tile([P, 1024], bf16, tag="kst")
                        nc.vector.tensor_copy(kst[:], ps[:])
                        nc.sync.dma_start(
                            kT_dram[m, :, half * 1024:(half + 1) * 1024], kst[:])

                # deferred AdaLN embedding groups (needed from LN2 on)
                emb_group(1)
                emb_group(2)

                # Q projection (own half, softmax scale pre-folded into wq1)
                qT = qTp.tile([P, KT, N_OWN], bf16, tag="qT")
                w_sb = wbig.tile([P, KT, INNER], bf16, tag="w")
                nc.sync.dma_start(w_sb[:], wq1_d[:])
                for m in range(KT):
                    ps = ps_big.tile([P, 1024], f32, tag="big")
                    for qc in range(2):
                        for kt in range(KT):
                            nc.tensor.matmul(
                                ps[:, qc * 512:(qc + 1) * 512],
                                w_sb[:, kt, m * P:(m + 1) * P],
                                h1T[:, kt, qc * 512:(qc + 1) * 512],
                                start=(kt == 0), stop=(kt == KT - 1))
                    nc.vector.tensor_copy(qT[:, m, :], ps[:])

            # h1T freed here.

            # ---------------- cross K2/V2 (early, PE filler) ----------------
            k2T = vp.tile([P, KT, J], bf16, tag="k2T")
            w_sb = wbig.tile([P, KT, INNER], bf16, tag="w")
            nc.sync.dma_start(w_sb[:, 0:CKT, :], wk2_d[:])
            for m in range(KT):
                ps = ps_av.tile([P, 512], f32, tag="av")
                for kt in range(CKT):
                    nc.tensor.matmul(ps[:, 0:J], w_sb[:, kt, m * P:(m + 1) * P],
                                     ctxT_sb[:, kt, :],
                                     start=(kt == 0), stop=(kt == CKT - 1))
                nc.vector.tensor_copy(k2T[:, m, :], ps[:, 0:J])

            v2_sb = vp.tile([P, J // P, H, DH + 1], bf16, tag="v2")
            nc.vector.memset(v2_sb[:], 1.0)
            w_sb = wbig.tile([P, KT, INNER], bf16, tag="w")
            nc.sync.dma_start(w_sb[:, 0:CKT, :], wv2_d[:])
            for tt in range(J // P):
                ps = ps_big.tile([P, 1024], f32, tag="big")
                for nc2 in range(2):
                    for kt in range(CKT):
                        nc.tensor.matmul(
                            ps[:, nc2 * 512:(nc2 + 1) * 512],
                            ctxT_sb[:, kt, tt * P:(tt + 1) * P],
                            w_sb[:, kt, nc2 * 512:(nc2 + 1) * 512],
                            start=(kt == 0), stop=(kt == CKT - 1))
                nc.vector.tensor_copy(
                    v2_sb[:, tt, :, 0:DH],
                    ps[:].rearrange("p (hh r) -> p hh r", r=DH))

            # ---------------- attention core ----------------
            with tc.tile_pool(name="expp", bufs=2) as expp, \
                 tc.tile_pool(name="atp", bufs=1) as atp, \
                 tc.tile_pool(name="kcp", bufs=2) as kcp:

                def attn_qc(qc, get_k, v_t, qT_t, nkt, out_T):
                    """All heads of one query-chunk of attention."""
                    qs = slice(qc * 512, (qc + 1) * 512)
                    for h in range(H):
                        hp = (h % 2) * 64
                        m2 = h // 2
                        kap = get_k(h)
                        exs = []
                        for half in range((nkt + 7) // 8):
                            ex = expp.tile([P, 8, 512], bf16, tag="ex")
                            exs.append(ex)
                        for kt2 in range((nkt + 1) // 2):
                            ps = ps_big.tile([P, 1024], f32, tag="big")
                            nkk = min(2, nkt)
                            for u in range(nkk):
                                kt = kt2 * 2 + u
                                nc.tensor.matmul(
                                    ps[:, u * 512:(u + 1) * 512],
                                    kap[hp:hp + 64, kt * P:(kt + 1) * P],
                                    qT_t[hp:hp + 64, m2, qs],
                                    start=True, stop=True)
                            ex = exs[kt2 // 4]
                            lo = (kt2 % 4) * 2
                            nc.scalar.activation(
                                ex[:, lo:lo + nkk, :]
                                .rearrange("p a b -> p (a b)"),
                                ps[:, 0:nkk * 512], AF.Exp)
                        pav = ps_av.tile([P, 512], f32, tag="av")
                        for kt in range(nkt):
                            nc.tensor.matmul(
                                pav[0:65], v_t[:, kt, h, :],
                                exs[kt // 8][:, kt % 8, :],
                                start=(kt == 0), stop=(kt == nkt - 1))
                        den = small.tile([1, 512], f32, tag="den")
                        nc.vector.tensor_copy(den[:], pav[64:65, :])
                        rec32 = small.tile([1, 512], f32, tag="den",
                                           name="rec32")
                        nc.vector.reciprocal_approx_fast(rec32[:], den[:])
                        bcs = small.tile([64, 512], f32, tag="bcs")
                        nc.gpsimd.partition_broadcast(bcs[:], rec32[:])
                        if hp == 0:
                            nc.vector.tensor_tensor(out_T[0:64, m2, qs],
                                                    pav[0:64], bcs[:], ALU.mult)
                        else:
                            tmp = small.tile([64, 512], bf16, tag="todd")
                            nc.vector.tensor_tensor(tmp[:], pav[0:64],
                                                    bcs[:], ALU.mult)
                            nc.sync.dma_start(out_T[64:128, m2, qs], tmp[:])

                def out_proj(attn_T, w_t, brow_i, resid_src, out_dram, tt,
                             ln_grp, h_dst):
                    """o-proj + bias + residual for token tile tt; LN into
                    h_dst."""
                    ps = ps_big.tile([P, 1024], f32, tag="big")
                    for dc in range(2):
                        dsl = slice(dc * 512, (dc + 1) * 512)
                        for m in range(KT):
                            nc.tensor.matmul(ps[:, dsl],
                                             attn_T[:, m, tt * P:(tt + 1) * P],
                                             w_t[:, m, dsl],
                                             start=(m == 0), stop=False)
                        nc.tensor.matmul(
                            ps[:, dsl], ones[0:1, :],
                            brow_sb[0:1, brow_i * D + dc * 512:
                                    brow_i * D + (dc + 1) * 512],
                            start=False, stop=True)
                    xot = stg.tile([P, D], f32, tag="ostage")
                    for dc in range(2):
                        dsl = slice(dc * 512, (dc + 1) * 512)
                        rt = xpool.tile([P, 512], f32, tag="x")
                        nc.sync.dma_start(rt[:], resid_src[tt, :, dsl])
                        nc.vector.tensor_tensor(xot[:, dsl], ps[:, dsl], rt[:],
                                                ALU.add)
                    nc.sync.dma_start(out_dram[tt], xot[:])
                    layernorm_tile(xot, ln_grp, h_dst, tt * P)

                # -------- self-attention + o1 + LN2, qc-interleaved --------
                attn1T = atp.tile([P, KT, N_OWN], bf16, tag="attnT")
                h2T = hTp.tile([P, KT, N_OWN], bf16, tag="hT")
                wo1_sb = wbig.tile([P, KT, INNER], bf16, tag="w")
                nc.sync.dma_start(wo1_sb[:], wo1_d[:])

                _kc = {}

                def get_k_self(h):
                    m2 = h // 2
                    if _kc.get("m2") != m2:
                        kth = kcp.tile([P, N], bf16, tag="kth")
                        nc.sync.dma_start(kth[:], kT_dram[m2])
                        _kc["m2"] = m2
                        _kc["t"] = kth
                    return _kc["t"]

                for qc in range(2):
                    _kc.clear()
                    attn_qc(qc, get_k_self, v_sb, qT, TT_FULL, attn1T)
                    for tt in range(qc * 4, qc * 4 + 4):
                        out_proj(attn1T, wo1_sb, 0, xf_d, x1_dram, tt, 1, h2T)
                    if qc == 0:
                        # prefetch q2 weights mid-attention (ring slot of wv2)
                        wq2_sb = wbig.tile([P, KT, INNER], bf16, tag="w")
                        nc.sync.dma_start(wq2_sb[:], wq2_d[:])

                # -------- q2 projection --------
                q2T = qTp.tile([P, KT, N_OWN], bf16, tag="qT")
                for m in range(KT):
                    ps = ps_big.tile([P, 1024], f32, tag="big")
                    for qc in range(2):
                        for kt in range(KT):
                            nc.tensor.matmul(
                                ps[:, qc * 512:(qc + 1) * 512],
                                wq2_sb[:, kt, m * P:(m + 1) * P],
                                h2T[:, kt, qc * 512:(qc + 1) * 512],
                                start=(kt == 0), stop=(kt == KT - 1))
                    nc.vector.tensor_copy(q2T[:, m, :], ps[:])

                # -------- cross-attention + o2 + LN3 --------
                attn2T = atp.tile([P, KT, N_OWN], bf16, tag="attnT")
                h3T = hTp.tile([P, KT, N_OWN], bf16, tag="hT")
                wo2_sb = wbig.tile([P, KT, INNER], bf16, tag="w")
                nc.sync.dma_start(wo2_sb[:], wo2_d[:])

                def get_k_cross(h):
                    return k2T[:, h // 2, :]

                for qc in range(2):
                    attn_qc(qc, get_k_cross, v2_sb, q2T, J // P, attn2T)
                    for tt in range(qc * 4, qc * 4 + 4):
                        out_proj(attn2T, wo2_sb, 1, x1_dram, x2_dram, tt, 2, h3T)

        # ---------------- Phase 6: GEGLU FF ----------------
        with tc.tile_pool(name="gp", bufs=1) as gp, \
             tc.tile_pool(name="wf2p", bufs=2) as wf2p:
            g_sb = gp.tile([P, 32, N_OWN], bf16, tag="g")
            for fc in range(32):
                wa = wsm.tile([P, KT, P], bf16, tag="wstream")
                nc.sync.dma_start(wa[:], wf1_d[:, :, fc * P:(fc + 1) * P])
                wg = wsm.tile([P, KT, P], bf16, tag="wstream")
                nc.sync.dma_start(wg[:], wf1_d[:, :, FF + fc * P:FF + (fc + 1) * P])
                for qc in range(2):
                    qs = slice(qc * 512, (qc + 1) * 512)
                    ps = ps_big.tile([P, 1024], f32, tag="big")
                    for kt in range(KT):
                        nc.tensor.matmul(ps[:, 0:512], wa[:, kt, :], h3T[:, kt, qs],
                                         start=(kt == 0), stop=(kt == KT - 1))
                    for kt in range(KT):
                        nc.tensor.matmul(ps[:, 512:1024], wg[:, kt, :],
                                         h3T[:, kt, qs],
                                         start=(kt == 0), stop=(kt == KT - 1))
                    gt = small.tile([P, 512], bf16, tag="gt")
                    nc.scalar.activation(gt[:], ps[:, 512:1024], AF.Gelu,
                                         bias=fb1_sb[:, 32 + fc:32 + fc + 1])
                    nc.vector.scalar_tensor_tensor(
                        g_sb[:, fc, qs], ps[:, 0:512], fb1_sb[:, fc:fc + 1],
                        gt[:], ALU.add, ALU.mult)

            for dc in range(4):
                dsl = slice(dc * 256, (dc + 1) * 256)
                wf2t = wf2p.tile([P, 32, 256], bf16, tag="wf2")
                nc.sync.dma_start(wf2t[:], wf2_d[:, :, dsl])
                for tt in range(TT_OWN):
                    ps = ps_av.tile([P, 512], f32, tag="av")
                    for j in range(32):
                        nc.tensor.matmul(ps[:, 0:256],
                                         g_sb[:, j, tt * P:(tt + 1) * P],
                                         wf2t[:, j, :],
                                         start=(j == 0), stop=False)
                    nc.tensor.matmul(ps[:, 0:256], ones[0:1, :],
                                     brow_sb[0:1, 2 * D + dc * 256:
                                             2 * D + (dc + 1) * 256],
                                     start=False, stop=True)
                    rt = xpool.tile([P, 512], f32, tag="x")
                    nc.sync.dma_start(rt[:, 0:256], x2_dram[tt, :, dsl])
                    yt = stg.tile([P, D], f32, tag="ostage")
                    nc.vector.tensor_tensor(yt[:, 0:256], ps[:, 0:256],
                                            rt[:, 0:256], ALU.add)
                    nc.sync.dma_start(y_d[tt, :, dsl], yt[:, 0:256])

    nc.compile()
    return nc


def _rearr_w(w, kt):
    return np.ascontiguousarray(
        w.reshape(kt, P, -1).transpose(1, 0, 2)).astype(BF16)


def _shard_inputs(inputs):
    f = {k: np.asarray(v, dtype=np.float32) for k, v in inputs.items()}
    shared = {
        "nw": _rearr_w(np.concatenate([f["n1_w"], f["n2_w"], f["n3_w"]], axis=1), KT),
        "nbc": np.ascontiguousarray(
            np.concatenate([f["n1_b"], f["n2_b"], f["n3_b"]])
            .reshape(3, 16, P).transpose(2, 0, 1).reshape(P, 48)),
        "wq1": _rearr_w(f["q1"] * SC, KT), "wk1": _rearr_w(f["k1"], KT),
        "wv1": _rearr_w(f["v1"], KT), "wo1": _rearr_w(f["o1_w"], KT),
        "wq2": _rearr_w(f["q2"] * SC, KT), "wk2": _rearr_w(f["k2"], CKT),
        "wv2": _rearr_w(f["v2"], CKT), "wo2": _rearr_w(f["o2_w"], KT),
        "brow": np.ascontiguousarray(
            np.concatenate([f["o1_b"], f["o2_b"], f["ff_b2"]])
            .reshape(1, 3 * D)).astype(BF16),
        "fb1c": np.ascontiguousarray(f["ff_b1"].reshape(64, P).T),
        "wf1": _rearr_w(f["ff_w1"], KT),
        "wf2": _rearr_w(f["ff_w2"], FF // P),
    }
    in_maps = []
    for core in range(8):
        b, half = core // 2, core % 2
        own = f["x"][b, half * N_OWN:(half + 1) * N_OWN]
        oth = f["x"][b, (1 - half) * N_OWN:(2 - half) * N_OWN]
        m = dict(shared)
        m["xf"] = np.ascontiguousarray(own.reshape(TT_OWN, P, D))
        m["xbf"] = np.ascontiguousarray(
            np.concatenate([own, oth]).reshape(TT_FULL, P, D)).astype(BF16)
        m["tT"] = np.ascontiguousarray(f["t"][b, 0].reshape(KT, P).T).astype(BF16)
        m["ctxT"] = np.ascontiguousarray(
            f["context"][b].T.reshape(CKT, P, J).transpose(1, 0, 2)).astype(BF16)
        in_maps.append(m)
    return in_maps


def kernel(**inputs):
    from concourse.bass_utils import run_bass_kernel_spmd
    if "nc" not in _CACHE:
        _CACHE["nc"] = _build_program()
    nc = _CACHE["nc"]
    in_maps = _shard_inputs(inputs)
    res = run_bass_kernel_spmd(nc, in_maps, core_ids=list(range(8)))
    out = np.empty((B, N, D), dtype=np.float32)
    for core in range(8):
        b, half = core // 2, core % 2
        out[b, half * N_OWN:(half + 1) * N_OWN] = \
            res.results[core]["y"].reshape(N_OWN, D)
    return out
